# revision 1
# baseline (speedup 1.0000x reference)
"""Trainium2 Bass kernel for nn_Dense_test (DH-SNN dense recurrent layer).

Contract: kernel(**inputs) takes the FULL unsharded inputs (as produced by
setup_inputs()) and returns the FULL [512, 3] float32 output.

Strategy (data-parallel over batch, 8 NeuronCores, 64 rows each):
  The model is a 2000-step leaky-integrate-and-fire recurrence:
      cur_t  = W1m @ [x_t, spk_{t-1}] + b1
      d_t    = beta*d_{t-1} + (1-beta)*cur_t
      mem1_t = alpha1*mem1_{t-1} + (1-alpha1)*d_t - spk_{t-1}
      spk_t  = (mem1_t > 1)
      mem2_t = alpha2*mem2_{t-1} + (1-alpha2)*(W2 @ spk_t + b2)
      out    = mean_t(mem2_t, t>=1)
  On-device layout is transposed (neurons on partitions, batch on the free
  dim) so every per-neuron coefficient is a per-partition scalar and each
  state update is a single fused scalar_tensor_tensor DVE op:
      D_t    = beta*D_{t-1} + P_t          with D == (1-alpha1)*d,
      P_t    = Weff @ [negspk_{t-1}; x_t; 1]   (weights row-scaled by
               (1-alpha1)(1-beta) on the host, Wh negated so the stored
               negative spikes {0,-1} contribute +Wh@spk)
      tmp_t  = alpha1*mem1_{t-1} + negspk_{t-1}
      mem1_t = tmp_t + D_t ; negspk_t = -(mem1_t > 1)  (exact in bf16)
  The readout W2 columns ride in the same stationary weights (psum rows
  96:99), so the 2-layer step needs only 4 bf16 matmuls (M=128 stationaries
  -> fast weight load), and x arrives pre-transposed from the host with a
  ones-row appended so b1 needs no extra work.
"""
import os
import numpy as np
import ml_dtypes

import concourse.bass as bass
import concourse.bacc as bacc
import concourse.mybir as mybir
import concourse.tile as tile
from concourse.bass_utils import run_bass_kernel_spmd

F32 = mybir.dt.float32
BF16 = mybir.dt.bfloat16
ALU = mybir.AluOpType

B_FULL = 512
T_FULL = 2000
H = 200
NIN = 32
O = 3
N_CORES = 8
BPC = B_FULL // N_CORES   # 64
K2 = 105                  # spk2(72) + x(32) + ones(1)
H2 = H - 128              # 72

_prog_cache = {}


def _build_program(T, CT):
    nc = bacc.Bacc("TRN2", target_bir_lowering=False, debug=False)

    x_in = nc.declare_dram_parameter("x", [T, 33, BPC], BF16, isOutput=False)
    # weights packed [128, 512]: cols 0:128 A1, 128:256 A2 (rows 0:K2),
    # 256:384 B1, 384:512 B2 (rows 0:K2)
    wts_d = nc.declare_dram_parameter("wts", [128, 512], BF16, isOutput=False)
    # scalars packed [128, 6]: col0 beta1, col1 al1a, col2 beta2(rows<72),
    # col3 al1b(rows<72), col4 al2(rows<3), col5 b2e(rows<3)
    sc_d = nc.declare_dram_parameter("sc", [128, 6], F32, isOutput=False)
    out_d = nc.declare_dram_parameter("out", [O, BPC], F32, isOutput=True)

    with tile.TileContext(nc) as tc:
        with (
            tc.tile_pool(name="const", bufs=1) as cpool,
            tc.tile_pool(name="state", bufs=1) as spool,
            tc.tile_pool(name="spk1", bufs=2) as s1pool,
            tc.tile_pool(name="stage", bufs=2) as stpool,
            tc.tile_pool(name="p1", bufs=2, space="PSUM") as p1pool,
            tc.tile_pool(name="p2", bufs=2, space="PSUM") as p2pool,
        ):
            # constants: 2 DMAs; a DVE touch anchors DVE-side ordering so
            # per-step instructions never carry more than one sync wait
            wts = cpool.tile([128, 512], BF16, tag="wts")
            sc = cpool.tile([128, 6], F32, tag="sc")
            nc.sync.dma_start(wts[:], wts_d[:])
            nc.sync.dma_start(sc[:], sc_d[:])
            nc.vector.tensor_scalar_mul(sc[:], sc[:], 1.0)
            A1 = wts[:, 0:128]
            A2 = wts[0:K2, 128:256]
            B1 = wts[:, 256:384]
            B2 = wts[0:K2, 384:512]
            beta1 = sc[:, 0:1]
            al1a = sc[:, 1:2]
            beta2 = sc[0:H2, 2:3]
            al1b = sc[0:H2, 3:4]
            al2 = sc[0:O, 4:5]
            b2e = sc[0:O, 5:6]

            D1 = spool.tile([128, BPC], F32, tag="D1")
            D2 = spool.tile([H2, BPC], F32, tag="D2")
            m1a = spool.tile([128, BPC], F32, tag="m1a")
            m1b = spool.tile([H2, BPC], F32, tag="m1b")
            tmp1 = spool.tile([128, BPC], F32, tag="tmp1")
            tmp2 = spool.tile([H2, BPC], F32, tag="tmp2")
            mem2 = spool.tile([O, BPC], F32, tag="mem2")
            acc = spool.tile([O, BPC], F32, tag="acc")
            for t_ in (D1, D2, m1a, m1b, mem2, acc):
                nc.vector.memset(t_[:], 0.0)

            # wide per-chunk tiles; col j of chunk c serves step t=c*CT+j:
            # spk cols hold negspk(t-1); stage rows 72:105 hold [x(t); 1]
            s1tiles = {}
            sttiles = {}

            def ensure_chunk(c):
                if c in s1tiles or c * CT >= T + 1:
                    return
                s1 = s1pool.tile([128, CT * BPC], BF16, tag="s1w")
                st = stpool.tile([K2, CT * BPC], BF16, tag="stw")
                s1tiles[c] = s1
                sttiles[c] = st
                t0 = c * CT
                nx = min(CT, T - t0)
                if nx > 0:
                    src = x_in[t0:t0 + nx, :, :].rearrange("t i b -> i t b")
                    dst = st[72:105, 0:nx * BPC].rearrange("i (t b) -> i t b", t=nx)
                    nc.sync.dma_start(dst, src)
                if c == 0:
                    nc.vector.memset(s1[:, 0:BPC], 0.0)
                    nc.vector.memset(st[0:H2, 0:BPC], 0.0)

            ensure_chunk(0)

            def col(tile_, t, np_=None):
                j = t % CT
                return tile_[0:(np_ or tile_.shape[0]), j * BPC:(j + 1) * BPC]

            for t in range(T):
                c = t // CT
                ensure_chunk(c)
                spk1_r = col(s1tiles[c], t)
                stage_r = col(sttiles[c], t)
                spk2_r = col(sttiles[c], t, H2)

                P1 = p1pool.tile([128, BPC], F32, tag="P1")
                P2 = p2pool.tile([128, BPC], F32, tag="P2")
                nc.tensor.matmul(P1[:], A1, spk1_r, start=True, stop=False)
                nc.tensor.matmul(P1[:], A2, stage_r, start=False, stop=True)
                nc.tensor.matmul(P2[:], B1, spk1_r, start=True, stop=False)
                nc.tensor.matmul(P2[:], B2, stage_r, start=False, stop=True)

                nc.vector.scalar_tensor_tensor(tmp1[:], m1a[:], al1a, spk1_r,
                                               ALU.mult, ALU.add)
                nc.vector.scalar_tensor_tensor(tmp2[:], m1b[:], al1b, spk2_r,
                                               ALU.mult, ALU.add)

                nc.vector.scalar_tensor_tensor(D1[:], D1[:], beta1, P1[:],
                                               ALU.mult, ALU.add)
                nc.vector.scalar_tensor_tensor(D2[:], D2[:], beta2, P2[0:H2, :],
                                               ALU.mult, ALU.add)
                nc.vector.tensor_tensor(m1a[:], tmp1[:], D1[:], ALU.add)
                nc.vector.tensor_tensor(m1b[:], tmp2[:], D2[:], ALU.add)

                if t + 1 < T:
                    cn = (t + 1) // CT
                    ensure_chunk(cn)
                    nc.vector.tensor_scalar(col(s1tiles[cn], t + 1), m1a[:],
                                            1.0, -1.0, ALU.is_gt, ALU.mult)
                    nc.vector.tensor_scalar(col(sttiles[cn], t + 1, H2), m1b[:],
                                            1.0, -1.0, ALU.is_gt, ALU.mult)

                nc.vector.scalar_tensor_tensor(mem2[:], mem2[:], al2,
                                               P2[96:96 + O, :], ALU.mult, ALU.add)
                nc.vector.tensor_scalar_add(mem2[:], mem2[:], b2e)
                if t >= 1:
                    nc.gpsimd.tensor_tensor(acc[:], acc[:], mem2[:], ALU.add)

            nc.sync.dma_start(out_d[:], acc[:])
    nc.compile()
    return nc


def _get_program(T, CT):
    key = (T, CT)
    if key not in _prog_cache:
        _prog_cache[key] = _build_program(T, CT)
    return _prog_cache[key]


def _sigmoid64(v):
    return 1.0 / (1.0 + np.exp(-np.asarray(v, np.float64)))


def _host_prep(inputs, T):
    x = np.asarray(inputs["x"], np.float32)
    W1 = np.asarray(inputs["W1"], np.float32)
    b1 = np.asarray(inputs["b1"], np.float32)
    tau_n = np.asarray(inputs["tau_n"], np.float32)
    tau_m1 = np.asarray(inputs["tau_m1"], np.float32)
    W2 = np.asarray(inputs["W2"], np.float32)
    b2 = np.asarray(inputs["b2"], np.float32)
    tau_m2 = np.asarray(inputs["tau_m2"], np.float32)
    mask = np.asarray(inputs["mask"], np.float32)

    beta = _sigmoid64(tau_n).astype(np.float32).reshape(H)
    alpha1 = _sigmoid64(tau_m1).astype(np.float32)
    alpha2 = _sigmoid64(tau_m2).astype(np.float32)
    Wm = (W1 * mask).astype(np.float32)
    Wx = Wm[:, :NIN]
    Wh = Wm[:, NIN:]
    s1 = ((1.0 - alpha1) * (1.0 - beta)).astype(np.float32)

    Wh_eff = (-(s1[:, None] * Wh)).astype(ml_dtypes.bfloat16)
    Wx_eff = (s1[:, None] * Wx).astype(ml_dtypes.bfloat16)
    b1_eff = (s1 * b1).astype(ml_dtypes.bfloat16)
    W2_eff = (-((1.0 - alpha2)[:, None] * W2)).astype(ml_dtypes.bfloat16)

    A1 = np.zeros((128, 128), ml_dtypes.bfloat16)
    A1[:, :] = Wh_eff[0:128, 0:128].T
    A2 = np.zeros((K2, 128), ml_dtypes.bfloat16)
    A2[0:H2, :] = Wh_eff[0:128, 128:H].T
    A2[H2:H2 + NIN, :] = Wx_eff[0:128, :].T
    A2[K2 - 1, :] = b1_eff[0:128]
    B1 = np.zeros((128, 128), ml_dtypes.bfloat16)
    B1[:, 0:H2] = Wh_eff[128:H, 0:128].T
    B1[:, 96:96 + O] = W2_eff[:, 0:128].T
    B2 = np.zeros((K2, 128), ml_dtypes.bfloat16)
    B2[0:H2, 0:H2] = Wh_eff[128:H, 128:H].T
    B2[H2:H2 + NIN, 0:H2] = Wx_eff[128:H, :].T
    B2[K2 - 1, 0:H2] = b1_eff[128:H]
    B2[0:H2, 96:96 + O] = W2_eff[:, 128:H].T

    wts = np.zeros((128, 512), ml_dtypes.bfloat16)
    wts[:, 0:128] = A1
    wts[0:K2, 128:256] = A2
    wts[:, 256:384] = B1
    wts[0:K2, 384:512] = B2
    sc = np.zeros((128, 6), np.float32)
    sc[:, 0] = beta[0:128]
    sc[:, 1] = alpha1[0:128]
    sc[0:H2, 2] = beta[128:H]
    sc[0:H2, 3] = alpha1[128:H]
    sc[0:O, 4] = alpha2
    sc[0:O, 5] = (1.0 - alpha2) * b2
    shared = dict(wts=wts, sc=sc)

    per_core_x = []
    for ci in range(N_CORES):
        xs = x[ci * BPC:(ci + 1) * BPC, :T, :]
        xt = np.empty((T, 33, BPC), np.float32)
        xt[:, 0:NIN, :] = np.transpose(xs, (1, 2, 0))
        xt[:, NIN, :] = 1.0
        per_core_x.append(xt.astype(ml_dtypes.bfloat16))

    B = x.shape[0]

    def postfn(outs):
        full = np.empty((B, O), np.float32)
        for ci in range(N_CORES):
            full[ci * BPC:(ci + 1) * BPC, :] = (
                np.asarray(outs[ci], np.float32).T / np.float32(T))
        return full

    return shared, per_core_x, postfn


def kernel_with_stats(trace=False, CT=64, **inputs):
    T = np.asarray(inputs["x"]).shape[1]
    nc = _get_program(T, CT)
    shared, per_core_x, postfn = _host_prep(inputs, T)
    in_maps = [dict(shared, x=per_core_x[ci]) for ci in range(N_CORES)]
    res = run_bass_kernel_spmd(nc, in_maps, list(range(N_CORES)), trace=trace)
    outs = [res.results[ci]["out"] for ci in range(N_CORES)]
    return postfn(outs), res


def kernel(**inputs):
    out, _ = kernel_with_stats(**inputs)
    return out


# revision 2
# speedup vs baseline: 1.0884x; 1.0884x over previous
"""Trainium2 Bass kernel for nn_Dense_test (DH-SNN dense recurrent layer).\n\nkernel(**inputs) takes the FULL unsharded inputs and returns the FULL\n[512, 3] float32 output. Data-parallel over batch: 8 NeuronCores x 64 rows,\nthe whole T=2000 recurrence runs on-chip per core.\n\nSee build_program docstring for the per-step device math.\n"""

import numpy as np
import ml_dtypes

import concourse.bass as bass
import concourse.bacc as bacc
import concourse.mybir as mybir
import concourse.tile as tile

F32 = mybir.dt.float32
BF16 = mybir.dt.bfloat16
ALU = mybir.AluOpType

H = 200
NIN = 32
O = 3
BPC = 64
K2 = 105
H2 = H - 128
G_TAIL = 1000


def build_program(T, CT=64, debug=False, x_T=None):
    nc = bacc.Bacc("TRN2", target_bir_lowering=False, debug=debug)
    tg = max(1, T - (G_TAIL - 1))   # G covers device steps tg..T

    x_in = nc.declare_dram_parameter("x", [x_T or T, 33, BPC], BF16, isOutput=False)
    wts_d = nc.declare_dram_parameter("wts", [128, 512], BF16, isOutput=False)
    # sc [128, 12]: beta1|al1a|beta2|al1b|al2|unused|Wa1f(3)|Wa2f(3)
    sc_d = nc.declare_dram_parameter("sc", [128, 12], F32, isOutput=False)
    oacc_d = nc.declare_dram_parameter("oacc", [O, BPC], F32, isOutput=True)
    og_d = nc.declare_dram_parameter("og", [O, BPC], F32, isOutput=True)
    ou0_d = nc.declare_dram_parameter("ou0", [O, BPC], F32, isOutput=True)

    with tile.TileContext(nc) as tc:
        with (
            tc.tile_pool(name="const", bufs=1) as cpool,
            tc.tile_pool(name="state", bufs=1) as spool,
            tc.tile_pool(name="spk1", bufs=2) as s1pool,
            tc.tile_pool(name="stage", bufs=2) as stpool,
            tc.tile_pool(name="p1", bufs=2, space="PSUM") as p1pool,
            tc.tile_pool(name="p2", bufs=2, space="PSUM") as p2pool,
            tc.tile_pool(name="pend", bufs=1, space="PSUM") as pendpool,
        ):
            wts = cpool.tile([128, 512], BF16, tag="wts")
            sc = cpool.tile([128, 12], F32, tag="sc")
            nc.sync.dma_start(wts[:], wts_d[:])
            nc.sync.dma_start(sc[:], sc_d[:])
            nc.vector.tensor_scalar_mul(sc[:], sc[:], 1.0)       # DVE anchor
            A1 = wts[:, 0:128]
            A2 = wts[0:K2, 128:256]
            B1 = wts[:, 256:384]
            B2 = wts[0:K2, 384:512]
            beta1 = sc[:, 0:1]
            al1a = sc[:, 1:2]
            beta2 = sc[0:H2, 2:3]
            al1b = sc[0:H2, 3:4]
            al2 = sc[0:O, 4:5]
            Wa1f = sc[:, 6:6 + O]
            Wa2f = sc[0:H2, 9:9 + O]

            D1 = spool.tile([128, BPC], F32, tag="D1")
            D2 = spool.tile([H2, BPC], F32, tag="D2")
            m1a = spool.tile([128, BPC], F32, tag="m1a")
            m1b = spool.tile([H2, BPC], F32, tag="m1b")
            tmp1 = spool.tile([128, BPC], F32, tag="tmp1")
            tmp2 = spool.tile([H2, BPC], F32, tag="tmp2")
            G = spool.tile([O, BPC], F32, tag="G")
            for t_ in (D1, D2, m1a, m1b, G):
                nc.vector.memset(t_[:], 0.0)
            ssum1 = spool.tile([128, BPC], F32, tag="ssum1")
            ssum2 = spool.tile([H2, BPC], F32, tag="ssum2")
            u0s1 = spool.tile([128, BPC], F32, tag="u0s1")
            u0s2 = spool.tile([H2, BPC], F32, tag="u0s2")
            nc.gpsimd.memset(ssum1[:], 0.0)
            nc.gpsimd.memset(ssum2[:], 0.0)

            s1tiles = {}
            sttiles = {}

            def ensure_chunk(c):
                if c in s1tiles or c * CT >= T + 1:
                    return
                s1 = s1pool.tile([128, CT * BPC], BF16, tag="s1w")
                st = stpool.tile([K2, CT * BPC], BF16, tag="stw")
                s1tiles[c] = s1
                sttiles[c] = st
                t0 = c * CT
                nx = min(CT, T - t0)
                if nx > 0:
                    src = x_in[t0:t0 + nx, :, :].rearrange("t i b -> i t b")
                    dst = st[72:105, 0:nx * BPC].rearrange("i (t b) -> i t b", t=nx)
                    nc.sync.dma_start(dst, src)
                if c == 0:
                    nc.vector.memset(s1[:, 0:BPC], 0.0)
                    nc.vector.memset(st[0:H2, 0:BPC], 0.0)
                if c == T // CT:
                    # col T (virtual step) x-rows are never DMA'd; zero the
                    # whole column before step T-1's spike write fills 0:72
                    j = T % CT
                    nc.vector.memset(st[:, j * BPC:(j + 1) * BPC], 0.0)

            ensure_chunk(0)

            def col(tile_, t, np_=None):
                j = t % CT
                return tile_[0:(np_ or tile_.shape[0]), j * BPC:(j + 1) * BPC]

            for t in range(T + 1):
                c = t // CT
                ensure_chunk(c)
                spk1_r = col(s1tiles[c], t)
                stage_r = col(sttiles[c], t)
                spk2_r = col(sttiles[c], t, H2)

                last = (t == T)
                P2 = p2pool.tile([128, BPC], F32, tag="P2")
                if not last:
                    P1 = p1pool.tile([128, BPC], F32, tag="P1")
                    nc.tensor.matmul(P1[:], A1, spk1_r, start=True, stop=False)
                    nc.tensor.matmul(P1[:], A2, stage_r, start=False, stop=True)
                nc.tensor.matmul(P2[:], B1, spk1_r, start=True, stop=False)
                nc.tensor.matmul(P2[:], B2, stage_r, start=False, stop=True)

                if not last:
                    ensure_chunk((t + 1) // CT)
                    # tmp = al1*m1 + negspk1 / - spk2pos  (early; DVE)
                    nc.vector.scalar_tensor_tensor(tmp1[:], m1a[:], al1a, spk1_r,
                                                   ALU.mult, ALU.add)
                    nc.vector.scalar_tensor_tensor(tmp2[:], m1b[:], al1b, spk2_r,
                                                   ALU.mult, ALU.subtract)
                    nc.vector.scalar_tensor_tensor(D1[:], D1[:], beta1, P1[:],
                                                   ALU.mult, ALU.add)
                    nc.vector.tensor_tensor(m1a[:], tmp1[:], D1[:], ALU.add)
                    nc.vector.tensor_scalar(col(s1tiles[(t + 1) // CT], t + 1),
                                            m1a[:], 1.0, -1.0, ALU.is_gt, ALU.mult)
                    nc.vector.scalar_tensor_tensor(D2[:], D2[:], beta2,
                                                   P2[0:H2, :], ALU.mult, ALU.add)
                    nc.vector.tensor_tensor(m1b[:], tmp2[:], D2[:], ALU.add)
                    nc.vector.tensor_scalar(col(sttiles[(t + 1) // CT], t + 1, H2),
                                            m1b[:], 1.0, None, ALU.is_gt)

                if t >= 1:
                    # POOL: spike sums over cols 1..T (negspk1 / +spk2)
                    nc.gpsimd.tensor_tensor(ssum1[:], ssum1[:], spk1_r, ALU.add)
                    nc.gpsimd.tensor_tensor(ssum2[:], ssum2[:], spk2_r, ALU.add)
                if t == 1:
                    nc.gpsimd.tensor_copy(u0s1[:], spk1_r)
                    nc.gpsimd.tensor_copy(u0s2[:], spk2_r)

                if t >= tg:
                    nc.vector.scalar_tensor_tensor(G[:], G[:], al2,
                                                   P2[96:96 + O, :],
                                                   ALU.mult, ALU.add)

            P_end = pendpool.tile([O, BPC], F32, tag="pend")
            P_u0 = pendpool.tile([O, BPC], F32, tag="pu0")
            nc.tensor.matmul(P_end[:], Wa1f, ssum1[:], start=True, stop=False)
            nc.tensor.matmul(P_end[:], Wa2f, ssum2[:], start=False, stop=True)
            nc.tensor.matmul(P_u0[:], Wa1f, u0s1[:], start=True, stop=False)
            nc.tensor.matmul(P_u0[:], Wa2f, u0s2[:], start=False, stop=True)
            oacc_s = spool.tile([O, BPC], F32, tag="oaccs")
            ou0_s = spool.tile([O, BPC], F32, tag="ou0s")
            nc.vector.tensor_copy(oacc_s[:], P_end[:])
            nc.vector.tensor_copy(ou0_s[:], P_u0[:])
            nc.sync.dma_start(oacc_d[:], oacc_s[:])
            nc.sync.dma_start(og_d[:], G[:])
            nc.sync.dma_start(ou0_d[:], ou0_s[:])
    nc.compile()
    return nc


def sigmoid64(v):
    return 1.0 / (1.0 + np.exp(-np.asarray(v, np.float64)))


def host_prep(inputs, T=None):
    x = np.asarray(inputs["x"], np.float32)
    B, Tfull, _ = x.shape
    if T is None:
        T = Tfull
    W1 = np.asarray(inputs["W1"], np.float32)
    b1 = np.asarray(inputs["b1"], np.float32)
    tau_n = np.asarray(inputs["tau_n"], np.float32)
    tau_m1 = np.asarray(inputs["tau_m1"], np.float32)
    W2 = np.asarray(inputs["W2"], np.float32)
    b2 = np.asarray(inputs["b2"], np.float32)
    tau_m2 = np.asarray(inputs["tau_m2"], np.float32)
    mask = np.asarray(inputs["mask"], np.float32)

    beta = sigmoid64(tau_n).astype(np.float32).reshape(H)
    alpha1 = sigmoid64(tau_m1).astype(np.float32)
    alpha2 = sigmoid64(tau_m2).astype(np.float32)
    Wm = (W1 * mask).astype(np.float32)
    Wx = Wm[:, :NIN]
    Wh = Wm[:, NIN:]
    s1c = ((1.0 - alpha1) * (1.0 - beta)).astype(np.float32)

    Wh_eff = (-(s1c[:, None] * Wh)).astype(ml_dtypes.bfloat16)
    Wx_eff = (s1c[:, None] * Wx).astype(ml_dtypes.bfloat16)
    b1_eff = (s1c * b1).astype(ml_dtypes.bfloat16)
    W2b = np.asarray(W2, np.float32).astype(ml_dtypes.bfloat16)
    W2bf = np.asarray(W2b, np.float32)
    W2_eff = (-((1.0 - alpha2)[:, None] * W2bf)).astype(ml_dtypes.bfloat16)

    A1 = np.zeros((128, 128), ml_dtypes.bfloat16)
    A1[:, :] = Wh_eff[0:128, 0:128].T
    A2 = np.zeros((K2, 128), ml_dtypes.bfloat16)
    A2[0:H2, :] = -Wh_eff[0:128, 128:H].T     # chunk2 spikes positive
    A2[H2:H2 + NIN, :] = Wx_eff[0:128, :].T
    A2[K2 - 1, :] = b1_eff[0:128]
    B1 = np.zeros((128, 128), ml_dtypes.bfloat16)
    B1[:, 0:H2] = Wh_eff[128:H, 0:128].T
    B1[:, 96:96 + O] = W2_eff[:, 0:128].T
    B2 = np.zeros((K2, 128), ml_dtypes.bfloat16)
    B2[0:H2, 0:H2] = -Wh_eff[128:H, 128:H].T
    B2[H2:H2 + NIN, 0:H2] = Wx_eff[128:H, :].T
    B2[K2 - 1, 0:H2] = b1_eff[128:H]
    B2[0:H2, 96:96 + O] = -W2_eff[:, 128:H].T

    wts = np.zeros((128, 512), ml_dtypes.bfloat16)
    wts[:, 0:128] = A1
    wts[0:K2, 128:256] = A2
    wts[:, 256:384] = B1
    wts[0:K2, 384:512] = B2
    sc = np.zeros((128, 12), np.float32)
    sc[:, 0] = beta[0:128]
    sc[:, 1] = alpha1[0:128]
    sc[0:H2, 2] = beta[128:H]
    sc[0:H2, 3] = alpha1[128:H]
    sc[0:O, 4] = alpha2
    # end-matmul weights (f32): ssum1 holds NEGATIVE counts, ssum2 positive
    sc[:, 6:6 + O] = -W2bf[:, 0:128].T
    sc[0:H2, 9:9 + O] = W2bf[:, 128:H].T
    shared = dict(wts=wts, sc=sc)

    n_cores = 8
    per_core_x = []
    for ci in range(n_cores):
        xs = x[ci * BPC:(ci + 1) * BPC, :T, :]
        xt = np.empty((T, 33, BPC), np.float32)
        xt[:, 0:NIN, :] = np.transpose(xs, (1, 2, 0))
        xt[:, NIN, :] = 1.0
        per_core_x.append(xt.astype(ml_dtypes.bfloat16))

    a2 = alpha2.astype(np.float64)
    tpow = np.arange(1, T)
    C0 = np.array([np.float64(b2[o]) * np.sum(1.0 - a2[o] ** (tpow + 1))
                   for o in range(O)])

    def postfn(outs):
        full = np.empty((B, O), np.float32)
        for ci in range(n_cores):
            oacc = np.asarray(outs[ci]["oacc"], np.float64)
            og = np.asarray(outs[ci]["og"], np.float64)
            ou0 = np.asarray(outs[ci]["ou0"], np.float64)
            outT = (oacc
                    - (a2 / (1 - a2))[:, None] * og
                    - (1 - a2)[:, None] * ou0
                    + C0[:, None])
            full[ci * BPC:(ci + 1) * BPC, :] = (outT.T / T).astype(np.float32)
        return full

    return shared, per_core_x, postfn



from concourse.bass_utils import run_bass_kernel_spmd

B_FULL = 512
T_FULL = 2000
N_CORES = 8

_prog_cache = {}


def _get_program(T, CT=64):
    key = (T, CT)
    if key not in _prog_cache:
        _prog_cache[key] = build_program(T, CT=CT)
    return _prog_cache[key]


def kernel_with_stats(trace=False, CT=64, **inputs):
    T = np.asarray(inputs["x"]).shape[1]
    nc = _get_program(T, CT)
    shared, per_core_x, postfn = host_prep(inputs, T)
    in_maps = [dict(shared, x=per_core_x[ci]) for ci in range(N_CORES)]
    res = run_bass_kernel_spmd(nc, in_maps, list(range(N_CORES)), trace=trace)
    outs = [res.results[ci] for ci in range(N_CORES)]
    return postfn(outs), res


def kernel(**inputs):
    out, _ = kernel_with_stats(**inputs)
    return out


# revision 3
# speedup vs baseline: 1.1214x; 1.0304x over previous
"""Trainium2 Bass kernel for nn_Dense_test (DH-SNN dense recurrent layer).\n\nkernel(**inputs) takes the FULL unsharded inputs and returns the FULL\n[512, 3] float32 output. Data-parallel over batch: 8 NeuronCores x 64 rows,\nthe whole T=2000 recurrence runs on-chip per core.\n"""

import numpy as np
import ml_dtypes

import concourse.bass as bass
import concourse.bacc as bacc
import concourse.mybir as mybir
import concourse.tile as tile

F32 = mybir.dt.float32
BF16 = mybir.dt.bfloat16
ALU = mybir.AluOpType

H = 200
NIN = 32
O = 3
BPC = 64
K2 = 105
H2 = H - 128
G_TAIL = 600


def build_program(T, CT=64, debug=False, x_T=None):
    nc = bacc.Bacc("TRN2", target_bir_lowering=False, debug=debug)
    tg = max(1, T - (G_TAIL - 1))   # G covers device steps tg..T

    x_in = nc.declare_dram_parameter("x", [x_T or T, 33, BPC], BF16, isOutput=False)
    wts_d = nc.declare_dram_parameter("wts", [128, 512], BF16, isOutput=False)
    # sc [128, 12]: beta1|al1a|beta2|al1b|al2|unused|Wa1f(3)|Wa2f(3)
    sc_d = nc.declare_dram_parameter("sc", [128, 12], F32, isOutput=False)
    oacc_d = nc.declare_dram_parameter("oacc", [O, BPC], F32, isOutput=True)
    og_d = nc.declare_dram_parameter("og", [O, BPC], F32, isOutput=True)
    ou0_d = nc.declare_dram_parameter("ou0", [O, BPC], F32, isOutput=True)

    with tile.TileContext(nc) as tc:
        with (
            tc.tile_pool(name="const", bufs=1) as cpool,
            tc.tile_pool(name="state", bufs=1) as spool,
            tc.tile_pool(name="spk1", bufs=2) as s1pool,
            tc.tile_pool(name="stage", bufs=2) as stpool,
            tc.tile_pool(name="p1", bufs=2, space="PSUM") as p1pool,
            tc.tile_pool(name="p2", bufs=2, space="PSUM") as p2pool,
            tc.tile_pool(name="pend", bufs=1, space="PSUM") as pendpool,
        ):
            wts = cpool.tile([128, 512], BF16, tag="wts")
            sc = cpool.tile([128, 12], F32, tag="sc")
            nc.sync.dma_start(wts[:], wts_d[:])
            nc.sync.dma_start(sc[:], sc_d[:])
            nc.vector.tensor_scalar_mul(sc[:], sc[:], 1.0)       # DVE anchor
            A1 = wts[:, 0:128]
            A2 = wts[0:K2, 128:256]
            B1 = wts[:, 256:384]
            B2 = wts[0:K2, 384:512]
            beta1 = sc[:, 0:1]
            al1a = sc[:, 1:2]
            beta2 = sc[0:H2, 2:3]
            al1b = sc[0:H2, 3:4]
            al2 = sc[0:O, 4:5]
            Wa1f = sc[:, 6:6 + O]
            Wa2f = sc[0:H2, 9:9 + O]

            # merged chunk tiles: cols 0:BPC = chunk1, BPC:2*BPC = chunk2
            Dt = spool.tile([128, 2 * BPC], F32, tag="Dt")
            mt = spool.tile([128, 2 * BPC], F32, tag="mt")
            tmpt = spool.tile([128, 2 * BPC], F32, tag="tmpt")
            G = spool.tile([O, BPC], F32, tag="G")
            for t_ in (Dt, mt, tmpt, G):
                nc.vector.memset(t_[:], 0.0)
            D1 = Dt[:, 0:BPC]
            D2 = Dt[0:H2, BPC:2 * BPC]
            m1a = mt[:, 0:BPC]
            m1b = mt[0:H2, BPC:2 * BPC]
            tmp1 = tmpt[:, 0:BPC]
            tmp2 = tmpt[0:H2, BPC:2 * BPC]
            ssum1 = spool.tile([128, BPC], F32, tag="ssum1")
            ssum2 = spool.tile([H2, BPC], F32, tag="ssum2")
            u0s1 = spool.tile([128, BPC], F32, tag="u0s1")
            u0s2 = spool.tile([H2, BPC], F32, tag="u0s2")
            nc.gpsimd.memset(ssum1[:], 0.0)
            nc.gpsimd.memset(ssum2[:], 0.0)

            s1tiles = {}
            sttiles = {}

            def ensure_chunk(c):
                if c in s1tiles or c * CT >= T + 1:
                    return
                s1 = s1pool.tile([128, CT * BPC], BF16, tag="s1w")
                st = stpool.tile([K2, CT * BPC], BF16, tag="stw")
                s1tiles[c] = s1
                sttiles[c] = st
                t0 = c * CT
                nx = min(CT, T - t0)
                if nx > 0:
                    src = x_in[t0:t0 + nx, :, :].rearrange("t i b -> i t b")
                    dst = st[72:105, 0:nx * BPC].rearrange("i (t b) -> i t b", t=nx)
                    nc.sync.dma_start(dst, src)
                if c == 0:
                    nc.vector.memset(s1[:, 0:BPC], 0.0)
                    nc.vector.memset(st[0:H2, 0:BPC], 0.0)
                if c == T // CT:
                    # col T (virtual step) x-rows are never DMA'd; zero the
                    # whole column before step T-1's spike write fills 0:72
                    j = T % CT
                    nc.vector.memset(st[:, j * BPC:(j + 1) * BPC], 0.0)

            ensure_chunk(0)

            def col(tile_, t, np_=None):
                j = t % CT
                return tile_[0:(np_ or tile_.shape[0]), j * BPC:(j + 1) * BPC]

            for t in range(T + 1):
                c = t // CT
                ensure_chunk(c)
                spk1_r = col(s1tiles[c], t)
                stage_r = col(sttiles[c], t)
                spk2_r = col(sttiles[c], t, H2)

                last = (t == T)
                P2 = p2pool.tile([128, BPC], F32, tag="P2")
                if not last:
                    P1 = p1pool.tile([128, BPC], F32, tag="P1")
                    nc.tensor.matmul(P1[:], A1, spk1_r, start=True, stop=False)
                    nc.tensor.matmul(P1[:], A2, stage_r, start=False, stop=True)
                nc.tensor.matmul(P2[:], B1, spk1_r, start=True, stop=False)
                nc.tensor.matmul(P2[:], B2, stage_r, start=False, stop=True)

                if not last:
                    ensure_chunk((t + 1) // CT)
                    # tmp = al1*m1 + negspk1 / - spk2pos  (early; DVE)
                    nc.vector.scalar_tensor_tensor(tmp1, m1a, al1a, spk1_r,
                                                   ALU.mult, ALU.add)
                    nc.vector.scalar_tensor_tensor(tmp2, m1b, al1b, spk2_r,
                                                   ALU.mult, ALU.subtract)
                    nc.vector.scalar_tensor_tensor(D1, D1, beta1, P1[:],
                                                   ALU.mult, ALU.add)
                    nc.vector.tensor_tensor(m1a, tmp1, D1, ALU.add)
                    nc.vector.tensor_scalar(col(s1tiles[(t + 1) // CT], t + 1),
                                            m1a, 1.0, -1.0, ALU.is_gt, ALU.mult)
                    nc.vector.scalar_tensor_tensor(D2, D2, beta2,
                                                   P2[0:H2, :], ALU.mult, ALU.add)
                    nc.vector.tensor_tensor(m1b, tmp2, D2, ALU.add)
                    nc.vector.tensor_scalar(col(sttiles[(t + 1) // CT], t + 1, H2),
                                            m1b, 1.0, None, ALU.is_gt)

                if t >= 1:
                    # POOL: spike sums over cols 1..T (negspk1 / +spk2)
                    nc.gpsimd.tensor_tensor(ssum1[:], ssum1[:], spk1_r, ALU.add)
                    nc.gpsimd.tensor_tensor(ssum2[:], ssum2[:], spk2_r, ALU.add)
                if t == 1:
                    nc.gpsimd.tensor_copy(u0s1[:], spk1_r)
                    nc.gpsimd.tensor_copy(u0s2[:], spk2_r)

                if t >= tg:
                    nc.vector.scalar_tensor_tensor(G[:], G[:], al2,
                                                   P2[96:96 + O, :],
                                                   ALU.mult, ALU.add)

            P_end = pendpool.tile([O, BPC], F32, tag="pend")
            P_u0 = pendpool.tile([O, BPC], F32, tag="pu0")
            nc.tensor.matmul(P_end[:], Wa1f, ssum1[:], start=True, stop=False)
            nc.tensor.matmul(P_end[:], Wa2f, ssum2[:], start=False, stop=True)
            nc.tensor.matmul(P_u0[:], Wa1f, u0s1[:], start=True, stop=False)
            nc.tensor.matmul(P_u0[:], Wa2f, u0s2[:], start=False, stop=True)
            oacc_s = spool.tile([O, BPC], F32, tag="oaccs")
            ou0_s = spool.tile([O, BPC], F32, tag="ou0s")
            nc.vector.tensor_copy(oacc_s[:], P_end[:])
            nc.vector.tensor_copy(ou0_s[:], P_u0[:])
            nc.sync.dma_start(oacc_d[:], oacc_s[:])
            nc.sync.dma_start(og_d[:], G[:])
            nc.sync.dma_start(ou0_d[:], ou0_s[:])
    nc.compile()
    return nc


def sigmoid64(v):
    return 1.0 / (1.0 + np.exp(-np.asarray(v, np.float64)))


def host_prep(inputs, T=None):
    x = np.asarray(inputs["x"], np.float32)
    B, Tfull, _ = x.shape
    if T is None:
        T = Tfull
    W1 = np.asarray(inputs["W1"], np.float32)
    b1 = np.asarray(inputs["b1"], np.float32)
    tau_n = np.asarray(inputs["tau_n"], np.float32)
    tau_m1 = np.asarray(inputs["tau_m1"], np.float32)
    W2 = np.asarray(inputs["W2"], np.float32)
    b2 = np.asarray(inputs["b2"], np.float32)
    tau_m2 = np.asarray(inputs["tau_m2"], np.float32)
    mask = np.asarray(inputs["mask"], np.float32)

    beta = sigmoid64(tau_n).astype(np.float32).reshape(H)
    alpha1 = sigmoid64(tau_m1).astype(np.float32)
    alpha2 = sigmoid64(tau_m2).astype(np.float32)
    Wm = (W1 * mask).astype(np.float32)
    Wx = Wm[:, :NIN]
    Wh = Wm[:, NIN:]
    s1c = ((1.0 - alpha1) * (1.0 - beta)).astype(np.float32)

    Wh_eff = (-(s1c[:, None] * Wh)).astype(ml_dtypes.bfloat16)
    Wx_eff = (s1c[:, None] * Wx).astype(ml_dtypes.bfloat16)
    b1_eff = (s1c * b1).astype(ml_dtypes.bfloat16)
    W2b = np.asarray(W2, np.float32).astype(ml_dtypes.bfloat16)
    W2bf = np.asarray(W2b, np.float32)
    W2_eff = (-((1.0 - alpha2)[:, None] * W2bf)).astype(ml_dtypes.bfloat16)

    A1 = np.zeros((128, 128), ml_dtypes.bfloat16)
    A1[:, :] = Wh_eff[0:128, 0:128].T
    A2 = np.zeros((K2, 128), ml_dtypes.bfloat16)
    A2[0:H2, :] = -Wh_eff[0:128, 128:H].T     # chunk2 spikes positive
    A2[H2:H2 + NIN, :] = Wx_eff[0:128, :].T
    A2[K2 - 1, :] = b1_eff[0:128]
    B1 = np.zeros((128, 128), ml_dtypes.bfloat16)
    B1[:, 0:H2] = Wh_eff[128:H, 0:128].T
    B1[:, 96:96 + O] = W2_eff[:, 0:128].T
    B2 = np.zeros((K2, 128), ml_dtypes.bfloat16)
    B2[0:H2, 0:H2] = -Wh_eff[128:H, 128:H].T
    B2[H2:H2 + NIN, 0:H2] = Wx_eff[128:H, :].T
    B2[K2 - 1, 0:H2] = b1_eff[128:H]
    B2[0:H2, 96:96 + O] = -W2_eff[:, 128:H].T

    wts = np.zeros((128, 512), ml_dtypes.bfloat16)
    wts[:, 0:128] = A1
    wts[0:K2, 128:256] = A2
    wts[:, 256:384] = B1
    wts[0:K2, 384:512] = B2
    sc = np.zeros((128, 12), np.float32)
    sc[:, 0] = beta[0:128]
    sc[:, 1] = alpha1[0:128]
    sc[0:H2, 2] = beta[128:H]
    sc[0:H2, 3] = alpha1[128:H]
    sc[0:O, 4] = alpha2
    # end-matmul weights (f32): ssum1 holds NEGATIVE counts, ssum2 positive
    sc[:, 6:6 + O] = -W2bf[:, 0:128].T
    sc[0:H2, 9:9 + O] = W2bf[:, 128:H].T
    shared = dict(wts=wts, sc=sc)

    n_cores = 8
    per_core_x = []
    for ci in range(n_cores):
        xs = x[ci * BPC:(ci + 1) * BPC, :T, :]
        xt = np.empty((T, 33, BPC), np.float32)
        xt[:, 0:NIN, :] = np.transpose(xs, (1, 2, 0))
        xt[:, NIN, :] = 1.0
        per_core_x.append(xt.astype(ml_dtypes.bfloat16))

    a2 = alpha2.astype(np.float64)
    tpow = np.arange(1, T)
    C0 = np.array([np.float64(b2[o]) * np.sum(1.0 - a2[o] ** (tpow + 1))
                   for o in range(O)])

    def postfn(outs):
        full = np.empty((B, O), np.float32)
        for ci in range(n_cores):
            oacc = np.asarray(outs[ci]["oacc"], np.float64)
            og = np.asarray(outs[ci]["og"], np.float64)
            ou0 = np.asarray(outs[ci]["ou0"], np.float64)
            outT = (oacc
                    - (a2 / (1 - a2))[:, None] * og
                    - (1 - a2)[:, None] * ou0
                    + C0[:, None])
            full[ci * BPC:(ci + 1) * BPC, :] = (outT.T / T).astype(np.float32)
        return full

    return shared, per_core_x, postfn



from concourse.bass_utils import run_bass_kernel_spmd

B_FULL = 512
T_FULL = 2000
N_CORES = 8

_prog_cache = {}


def _get_program(T, CT=64):
    key = (T, CT)
    if key not in _prog_cache:
        _prog_cache[key] = build_program(T, CT=CT)
    return _prog_cache[key]


def kernel_with_stats(trace=False, CT=64, **inputs):
    T = np.asarray(inputs["x"]).shape[1]
    nc = _get_program(T, CT)
    shared, per_core_x, postfn = host_prep(inputs, T)
    in_maps = [dict(shared, x=per_core_x[ci]) for ci in range(N_CORES)]
    res = run_bass_kernel_spmd(nc, in_maps, list(range(N_CORES)), trace=trace)
    outs = [res.results[ci] for ci in range(N_CORES)]
    return postfn(outs), res


def kernel(**inputs):
    out, _ = kernel_with_stats(**inputs)
    return out
